# revision 26
# baseline (speedup 1.0000x reference)
"""Distributed multi-head attention (RoPE) kernel for 8 TRN2 NeuronCores.

Sharding: batch x head-group. Core c handles batch c//4 and heads
4*(c%4) .. 4*(c%4)+3 (4 of 16 heads). Each core computes its heads'
QKV projections (column-sharded), RoPE, attention, and a row-sharded
partial output projection; the host sums the 4 partials per batch.

Self-contained: only needs numpy/ml_dtypes + the concourse stack at
/opt/trn_rl_repo (the runtime environment of this container).
"""

import contextlib
import sys

for _p in ("/opt/trn_rl_repo",):
    if _p not in sys.path:
        sys.path.insert(0, _p)

import numpy as np
import ml_dtypes

import concourse.bass as bass
import concourse.mybir as mybir
import concourse.tile as tile
from concourse.bass_utils import run_bass_kernel_spmd

F32 = mybir.dt.float32
BF16 = mybir.dt.bfloat16
BF = ml_dtypes.bfloat16

B, S_FULL, D_FULL, H_FULL, HD = 2, 4096, 1024, 16, 64
NH = 4  # heads per core
BASE = 10000.0


def _patch_tile_drain():
    """The walrus build in this container rejects >1 sem wait on one CTRL
    instruction ("Too many sync wait commands"). Split the Tile exit
    drain's waits across multiple drain instructions."""
    from concourse.tile import ScopedClock, TileContext

    if getattr(TileContext, "_drain_patched", False):
        return
    MAXW = 1

    def _drain_and_barrier(self, tick_clock, wait_clock):
        nc = self.nc
        drain_inst = nc.sync.drain()
        wait_clock.add_sem_waits(
            drain_inst.ins, ScopedClock({None: tick_clock.global_clock})
        )
        si = drain_inst.ins.sync_info
        waits = list(si.on_wait)
        if len(waits) > MAXW:
            drain_inst.ins.sync_info = mybir.SyncInfo(
                on_wait=waits[:MAXW], on_update=list(si.on_update)
            )
            for i in range(MAXW, len(waits), MAXW):
                extra = nc.sync.drain()
                extra.ins.sync_info = mybir.SyncInfo(
                    on_wait=waits[i : i + MAXW], on_update=[]
                )
        nc.all_engine_barrier()
        assert self.sems is not None
        popped = nc._tile_sem_poison_stack.pop()
        assert popped is self._sem_poison
        nc.clear_and_free_semaphores(list(self.sems.allocated().values()))
        nc.all_engine_barrier()

    TileContext._drain_and_barrier = _drain_and_barrier

    # The same limit applies to every instruction type: split surplus sem
    # waits onto same-engine NoOps emitted just before the instruction.
    _orig_commit = TileContext._commit_and_lower
    _ctr = [0]

    def _commit_and_lower(self, inst, bb, old_bb_map, bb_to_exit_bb):
        si = getattr(inst, "sync_info", None)
        eng = getattr(inst, "engine", None)
        mx = (
            2
            if type(inst).__name__ in getattr(tile, "_WS_MAXW2_TYPES", ())
            else MAXW
        )
        if (
            si is not None
            and len(si.on_wait) > mx
            and eng is not None
            and eng != mybir.EngineType.Unassigned
        ):
            waits = list(si.on_wait)
            for w in waits[:-mx]:
                _ctr[0] += 1
                nop = mybir.InstNoOp(name=f"WS-{_ctr[0]}", ins=[], outs=[])
                nop.engine = eng
                nop.sync_info = mybir.SyncInfo(on_wait=[w], on_update=[])
                self.nc.register_instruction(nop, overwrite=True)
                bb.add_instruction(nop)
            inst.sync_info = mybir.SyncInfo(
                on_wait=waits[-mx:], on_update=list(si.on_update)
            )
        return _orig_commit(self, inst, bb, old_bb_map, bb_to_exit_bb)

    TileContext._commit_and_lower = _commit_and_lower
    TileContext._drain_patched = True


_patch_tile_drain()


WS_MAXW2_TYPES = ()


def build_graph(S, DX, DO, qk_bias=False):
    """One core's graph. Layouts (T = [dims, seq]):
      inputs xq/xk/xv: X.T [DX, S]; weights wq/wk: permuted-rows W.T [DX, 256]
      (columns ordered re(128)|im(128) across the 4 local heads);
      wv: W.T [DX, 256] natural head order; wo: Wo[:, cols].T [256, DO].
      cos/sin tables [128, S] (row 32*hl+j = freq j of local head hl);
      Q tables pre-scaled by 1/8 (the 1/sqrt(HD) softmax scale).
    Output "out": partial x @ Wo_c.T, [S, DO] bf16.
    """
    DL = NH * HD  # 256
    KT = DX // 128
    SGW = min(1024, S)
    QC = 512
    OC = min(512, DO)
    NSG = S // SGW
    NQB = S // QC
    NKT = S // 128
    NST = S // 128
    NI = NH * NQB
    MUL, ADD, SUB = mybir.AluOpType.mult, mybir.AluOpType.add, mybir.AluOpType.subtract

    tile._WS_MAXW2_TYPES = WS_MAXW2_TYPES
    nc = bass.Bass(target_bir_lowering=False)
    ext = {}
    for nm, shp, dt in [
        ("xq", [DX, S], BF16), ("xk", [DX, S], BF16), ("xv", [DX, S], BF16),
        ("wq", [DX, DL], BF16), ("wk", [DX, DL], BF16), ("wv", [DX, DL], BF16),
        ("wo", [DL, DO], BF16),
        ("cosq", [128, S], BF16), ("sinq", [128, S], BF16),
        ("cosk", [128, S], BF16), ("sink", [128, S], BF16),
    ]:
        ext[nm] = nc.declare_dram_parameter(nm, shp, dt, isOutput=False)
    if qk_bias:
        ext["bq"] = nc.declare_dram_parameter("bq", [DL, 1], F32, isOutput=False)
        ext["bk"] = nc.declare_dram_parameter("bk", [DL, 1], F32, isOutput=False)
    out_ext = nc.declare_dram_parameter("out0", [S, DO], BF16, isOutput=True)
    scratch = nc.dram_tensor("scratch_recip", [NI, QC], F32)
    scratch2 = nc.dram_tensor("scratch_sums", [NI, QC], F32)

    with tile.TileContext(nc) as tc, contextlib.ExitStack() as top:
        pers = top.enter_context(tc.tile_pool(name="pers", bufs=1))
        QTh = [pers.tile([128, S], BF16, tag=f"qt{i}", name=f"qt{i}") for i in range(2)]
        KTh = [pers.tile([128, S], BF16, tag=f"kt{i}", name=f"kt{i}") for i in range(2)]
        CXh = [pers.tile([128, S], BF16, tag=f"cx{i}", name=f"cx{i}") for i in range(2)]
        VP = [pers.tile([128, NH * 65], BF16, tag=f"vp{i}", name=f"vp{i}") for i in range(NST)]
        NTS = (NI + 3) // 4
        sum_t = [pers.tile([97, QC], F32, tag=f"sums{i}", name=f"sums{i}") for i in range(NTS)]
        for i in range(NTS):
            nc.vector.memset(sum_t[i][:, :], 1.0)
        wq_t = [pers.tile([128, DL], BF16, tag=f"wq{k}", name=f"wq{k}") for k in range(KT)]
        wk_t = [pers.tile([128, DL], BF16, tag=f"wk{k}", name=f"wk{k}") for k in range(KT)]
        wv_t = [pers.tile([128, DL], BF16, tag=f"wv{k}", name=f"wv{k}") for k in range(KT)]
        wo_t = [pers.tile([128, DO], BF16, tag=f"wo{k}", name=f"wo{k}") for k in range(DL // 128)]
        for k in range(KT):
            for w_t, nm in ((wv_t, "wv"), (wk_t, "wk"), (wq_t, "wq")):
                eng = nc.gpsimd if nm != "wv" else nc.sync
                eng.dma_start(
                    out=w_t[k][:, :], in_=ext[nm][k * 128 : (k + 1) * 128, :]
                )
        for k in range(DL // 128):
            nc.gpsimd.dma_start(
                out=wo_t[k][:, :], in_=ext["wo"][k * 128 : (k + 1) * 128, :]
            )
        bias_t = {}
        if qk_bias:
            for nm in ("bq", "bk"):
                bias_t[nm] = pers.tile([DL, 1], F32, tag=nm, name=nm+"_t")
                # [DL,1] spans 2 partition tiles; DMA per 128-partition block
                for m in range(DL // 128):
                    nc.sync.dma_start(
                        out=bias_t[nm][m * 128 : (m + 1) * 128, :],
                        in_=ext[nm][m * 128 : (m + 1) * 128, :],
                    )

        # ---- projections (V, then K, then Q) in one pool scope so Tile
        # can overlap across the phase boundaries; all projection psums share
        # one 4-slot ring of [128, SGW] tiles (8 banks). ----
        with tc.tile_pool(name="qkps", bufs=4, space="PSUM") as psqk, \
             tc.tile_pool(name="qksb", bufs=KT + 12) as sbs, \
             tc.tile_pool(name="tabs", bufs=1) as tabs, \
             tc.tile_pool(name="rope", bufs=2) as rp:
            psvp, vsb = psqk, sbs
            # PE warmup: dummy matmuls with no DMA deps run during the initial
            # input DMA wait, flipping HAM to K=8/8 before real matmuls start.
            wsb = tabs.tile([128, 512], BF16, tag="warm")
            nc.vector.memset(wsb[:, :], 0.0)
            wps = psqk.tile([128, 512], F32, tag="qkps", name="warm_ps",
                            padded_shape=[128, SGW])
            for _w in range(40):
                nc.tensor.matmul(
                    wps[:, :], wsb[:, 0:128], wsb[:, :],
                    start=True, stop=True,
                )
            # cos/sin tables: issue early so their 4 MB of DMA spreads across
            # the V-projection phase instead of colliding with Q/K x loads.
            tabt = {}
            for nm in ("cosq", "sinq", "cosk", "sink"):
                tabt[nm] = tabs.tile([128, S], BF16, tag=nm, name=nm+"_t")
                nc.gpsimd.dma_start(out=tabt[nm][:, :], in_=ext[nm][:, :])
            for stg in range(NSG):
                xs = []
                for k in range(KT):
                    t = vsb.tile([128, SGW], BF16, tag="xs")
                    eng = (nc.sync, nc.scalar, nc.gpsimd)[k % 3]
                    eng.dma_start(
                        out=t[:, :],
                        in_=ext["xv"][k * 128 : (k + 1) * 128, stg * SGW : (stg + 1) * SGW],
                    )
                    xs.append(t)
                for st8 in range(SGW // 128):
                    st = stg * (SGW // 128) + st8
                    psv = psvp.tile([128, DL], F32, tag="qkps", name="psv", padded_shape=[128, SGW])
                    for k in range(KT):
                        nc.tensor.matmul(
                            psv[:, :],
                            xs[k][:, st8 * 128 : (st8 + 1) * 128],
                            wv_t[k][:, :],
                            start=(k == 0),
                            stop=(k == KT - 1),
                        )
                    vp3 = VP[st][:, :].rearrange("p (h c) -> p h c", c=65)
                    nc.vector.tensor_copy(
                        vp3[:, :, 0:64],
                        psv[:, :].rearrange("p (h c) -> p h c", c=64),
                    )
                    nc.vector.memset(vp3[:, :, 64:65], 1.0)

            # ---- Q/K projections + RoPE + head-major rearrange ----
            for xnm, w_t, dst, cnm, snm, bnm in (
                ("xq", wq_t, QTh, "cosq", "sinq", "bq"),
                ("xk", wk_t, KTh, "cosk", "sink", "bk"),
            ):
                cosT, sinT = tabt[cnm], tabt[snm]
                for sg in range(NSG):
                    ssl = slice(sg * SGW, (sg + 1) * SGW)
                    xs = []
                    for k in range(KT):
                        t = sbs.tile([128, SGW], BF16, tag="xs")
                        eng = nc.sync if k % 2 == 0 else nc.scalar
                        eng.dma_start(
                            out=t[:, :], in_=ext[xnm][k * 128 : (k + 1) * 128, ssl]
                        )
                        xs.append(t)
                    ps = [psqk.tile([128, SGW], F32, tag="qkps", name=f"qkps{_i}") for _i in range(2)]
                    for m in range(2):
                        for k in range(KT):
                            for qc in range(SGW // 512):
                                qsl = slice(qc * 512, (qc + 1) * 512)
                                nc.tensor.matmul(
                                    ps[m][:, qsl],
                                    w_t[k][:, m * 128 : (m + 1) * 128],
                                    xs[k][:, qsl],
                                    start=(k == 0),
                                    stop=(k == KT - 1),
                                )
                        if qk_bias:
                            nc.vector.tensor_scalar_add(
                                ps[m][:, :], ps[m][:, :],
                                bias_t[bnm][m * 128 : (m + 1) * 128, :],
                            )
                    a = rp.tile([128, SGW], F32, tag="ra")
                    b2 = rp.tile([128, SGW], F32, tag="rb")
                    ren = rp.tile([128, SGW], BF16, tag="ren")
                    imn = rp.tile([128, SGW], BF16, tag="imn")
                    nc.vector.tensor_tensor(out=a[:, :], in0=ps[0][:, :], in1=cosT[:, ssl], op=MUL)
                    nc.vector.tensor_tensor(out=b2[:, :], in0=ps[1][:, :], in1=sinT[:, ssl], op=MUL)
                    nc.vector.tensor_tensor(out=ren[:, :], in0=a[:, :], in1=b2[:, :], op=SUB)
                    nc.vector.tensor_tensor(out=a[:, :], in0=ps[0][:, :], in1=sinT[:, ssl], op=MUL)
                    nc.vector.tensor_tensor(out=b2[:, :], in0=ps[1][:, :], in1=cosT[:, ssl], op=MUL)
                    nc.vector.tensor_tensor(out=imn[:, :], in0=a[:, :], in1=b2[:, :], op=ADD)
                    # head-major rearrange via SBUF->SBUF DMA: pure partition
                    # moves, so they run on the DMA fabric instead of eating
                    # ~2.6us of DVE per (input, stage) in partition-sliced
                    # copies (a [32, S] copy costs as much as [128, S] on DVE).
                    for hl in range(NH):
                        pt, off = hl // 2, 64 * (hl % 2)
                        hsl = slice(32 * hl, 32 * hl + 32)
                        e1, e2 = ((nc.gpsimd, nc.scalar), (nc.sync, nc.gpsimd))[hl % 2]
                        e1.dma_start(
                            out=dst[pt][off : off + 32, ssl], in_=ren[hsl, :]
                        )
                        e2.dma_start(
                            out=dst[pt][off + 32 : off + 64, ssl], in_=imn[hsl, :]
                        )

        # ---- attention + incremental normalization + per-hp O-projection.
        # scoresT = K_h @ Q_h.T per 128-k-tile (the two heads' matmuls run
        # concurrently via row-group tiling); exp on ACT for 20 of every 32
        # k-tiles and bf16-Schraudolph on DVE for the other 12 (the attention
        # phase is otherwise ACT-exp serialized); outT[h] accumulates
        # [V_h|1].T @ Pexp over k. After each (hp, qb): stage ctxT, reciprocal
        # the sums row, DMA round-trip broadcast it and normalize in place.
        # After both hp: the partial output projection ctx @ Wo_c -> out.
        # Schraudolph bf16: bits16 = round(x * 128/ln2 + (16256 - 7.25));
        # DVE f32->int16 convert is round-to-nearest (verified on HW), the
        # -7.25 centers the relative error at zero so softmax denominators
        # cancel most of it. RMS rel err ~1.8% on the DVE half of tiles.
        #
        # Software pipeline: whole-tile exp ([128, 2*QC], both heads in one
        # instruction) alternating between ACT and the DVE (bf16 Schraudolph,
        # 7 of every 16 slots) -- the ~185ns bank-free -> scores-refill ->
        # exp handoff is paid per exp INSTRUCTION, so whole-tile halves it
        # versus per-head exps. The scores pair for slot i+2 is issued
        # BEFORE slot i's AV matmuls: both unblock when exp(i) frees its
        # banks, and scores-first ordering lets exp(i+2) start ~650ns
        # earlier -- with plain ordering the chain
        # exp(i) -> AV(i) -> scores(i+2) -> exp(i+2) serializes each exp
        # engine behind Tensor work. Normalization for a finished qb is
        # deferred into the next qb and spread over 4 slots, with all its
        # DMAs on the (attention-idle) sync queue, so its round-trip waits
        # never head-of-line-block the strict-FIFO DVE queue.
        SCH_A16 = 184.6649652337873  # 2^7 / ln 2
        SCH_B16 = 16256.0 - 7.25
        # Tensor binds the attention cadence (~920ns/slot), so ACT only needs
        # to cover 12 of 16 exps (12*1146/16 = 860ns/slot): keep the less
        # accurate DVE Schraudolph at 4 of 16 slots (rel err ~1.2e-2 total).
        DVE_I16 = (1, 5, 9, 13)  # i%16 slots whose exp runs on DVE
        with tc.tile_pool(name="aps", bufs=2, space="PSUM") as aps, \
             tc.tile_pool(name="ops", bufs=4, space="PSUM") as ops, \
             tc.tile_pool(name="pex", bufs=6) as pex, \
             tc.tile_pool(name="nrm", bufs=4) as nrm, \
             tc.tile_pool(name="osb", bufs=4) as osb:
            flat = [
                (hp, qb, kt)
                for hp in range(2)
                for qb in range(NQB)
                for kt in range(NKT)
            ]
            pso_of = {}
            pss_q = {}

            def emit_scores(i):
                hp, qb, kt = flat[i]
                qsl = slice(qb * QC, (qb + 1) * QC)
                ksl = slice(kt * 128, (kt + 1) * 128)
                pss = aps.tile([128, 2 * QC], F32, tag="scoresT")
                for h in range(2):
                    hoff = 64 * h
                    nc.tensor.matmul(
                        pss[:, h * QC : (h + 1) * QC],
                        KTh[hp][hoff : hoff + 64, ksl],
                        QTh[hp][hoff : hoff + 64, qsl],
                        start=True,
                        stop=True,
                    )
                pss_q[i] = pss

            def norm_phase1(hp, qb, state):
                # stage the exp-sum rows out of PSUM and kick the DMA chain
                # (scratch2 write then r4 read are FIFO-ordered on gpsimd);
                # h0's copy on Scalar, h1's on Vector to balance the load
                pso = pso_of[(hp, qb)]
                r4s = []
                for h in range(2):
                    hl = 2 * hp + h
                    ins = hl * NQB + qb
                    r = 32 * (ins % 4)
                    srow = sum_t[ins // 4][r : r + 1, :]
                    if h == 0:
                        nc.scalar.copy(srow, pso[h][64:65, :])
                    else:
                        nc.vector.tensor_copy(srow, pso[h][64:65, :])
                    nc.sync.dma_start(out=scratch2[ins : ins + 1, :], in_=srow)
                    r4 = nrm.tile([128, QC // 128], F32, tag="r4")
                    nc.sync.dma_start(
                        out=r4[:, :],
                        in_=scratch2[ins : ins + 1, :].rearrange(
                            "1 (p f) -> p f", p=128
                        ),
                    )
                    r4s.append(r4)
                state["r4"] = r4s

            def norm_phase2(hp, qb, state):
                for h in range(2):
                    hl = 2 * hp + h
                    ins = hl * NQB + qb
                    rc4 = nrm.tile([128, QC // 128], F32, tag="rc4")
                    nc.vector.reciprocal(rc4[:, :], state["r4"][h][:, :])
                    nc.sync.dma_start(
                        out=scratch[ins : ins + 1, :].rearrange(
                            "1 (p f) -> p f", p=128
                        ),
                        in_=rc4[:, :],
                    )
                bc = nrm.tile([128, QC], F32, tag="bc")
                for h in range(2):
                    hl = 2 * hp + h
                    ins = hl * NQB + qb
                    nc.sync.dma_start(
                        out=bc[64 * h : 64 * h + 64, :],
                        in_=scratch[ins : ins + 1, :].broadcast_to([64, QC]),
                    )
                state["bc"] = bc

            def norm_phase3(hp, qb, state, h):
                qsl = slice(qb * QC, (qb + 1) * QC)
                pso = pso_of[(hp, qb)]
                bc = state["bc"]
                nc.vector.tensor_tensor(
                    out=CXh[hp][64 * h : 64 * h + 64, qsl],
                    in0=pso[h][0:64, :],
                    in1=bc[64 * h : 64 * h + 64, :],
                    op=MUL,
                )
                if h == 1:
                    pso_of.pop((hp, qb))

            emit_scores(0)
            emit_scores(1)
            prev_qb = None
            norm_state = {}
            for i, (hp, qb, kt) in enumerate(flat):
                if kt == 0:
                    pso_of[(hp, qb)] = [
                        ops.tile([65, QC], F32, tag="outT", name=f"outT{_i}")
                        for _i in range(2)
                    ]
                if i + 2 < len(flat):
                    emit_scores(i + 2)
                pss = pss_q.pop(i)
                pe = pex.tile([128, 2 * QC], BF16, tag="pexp")
                if i % 16 in DVE_I16:
                    nc.vector.tensor_scalar(
                        pe[:, :].bitcast(mybir.dt.int16), pss[:, :],
                        SCH_A16, SCH_B16,
                        op0=MUL, op1=ADD,
                    )
                else:
                    nc.scalar.activation(
                        pe[:, :], pss[:, :],
                        mybir.ActivationFunctionType.Exp,
                    )
                for h in range(2):
                    hl = 2 * hp + h
                    nc.tensor.matmul(
                        pso_of[(hp, qb)][h][:, :],
                        VP[kt][:, 65 * hl : 65 * hl + 65],
                        pe[:, h * QC : (h + 1) * QC],
                        start=(kt == 0),
                        stop=(kt == NKT - 1),
                    )
                # deferred normalization of the previous qb
                if prev_qb is not None:
                    if kt == 1:
                        norm_phase1(*prev_qb, norm_state)
                    elif kt == 4:
                        norm_phase2(*prev_qb, norm_state)
                    elif kt == 8:
                        norm_phase3(*prev_qb, norm_state, 0)
                    elif kt == 10:
                        norm_phase3(*prev_qb, norm_state, 1)
                if kt == NKT - 1:
                    prev_qb = (hp, qb)
            norm_phase1(*prev_qb, norm_state)
            norm_phase2(*prev_qb, norm_state)
            norm_phase3(*prev_qb, norm_state, 0)
            norm_phase3(*prev_qb, norm_state, 1)
            # O-projection: out = ctx @ Wo_c.T, accumulating both head
            # pairs in PSUM -> single output. mt-outer so only the final
            # s-tiles wait on the last qb's normalization; both DO-halves of
            # an s-tile merge into one SBUF tile -> one out DMA per s-tile.
            # PSUM->SBUF staging alternates Vector/Scalar (the tail is
            # staging-bound, both engines are idle here); out DMA alternates
            # sync/gpsimd queues.
            for mt in range(NST):
                ob = osb.tile([128, DO], BF16, tag="ob")
                for n in range(DO // OC):
                    nsl = slice(n * OC, (n + 1) * OC)
                    po = ops.tile([128, OC], F32, tag="outT", name="po")
                    for k2 in range(2):
                        nc.tensor.matmul(
                            po[:, :],
                            CXh[k2][:, mt * 128 : (mt + 1) * 128],
                            wo_t[k2][:, nsl],
                            start=(k2 == 0),
                            stop=(k2 == 1),
                        )
                    if n % 2 == 0:
                        nc.vector.tensor_copy(ob[:, nsl], po[:, :])
                    else:
                        nc.scalar.copy(ob[:, nsl], po[:, :])
                eng = nc.sync if mt % 2 == 0 else nc.gpsimd
                eng.dma_start(
                    out=out_ext[mt * 128 : (mt + 1) * 128, :],
                    in_=ob[:, :],
                )
    return nc


def _perm_reim():
    """Permutation of a 256-row local head block: [re rows (128), im rows]."""
    re = [64 * hl + 2 * j for hl in range(NH) for j in range(HD // 2)]
    im = [64 * hl + 2 * j + 1 for hl in range(NH) for j in range(HD // 2)]
    return np.array(re + im)


def _tables(S, scale):
    j = np.arange(HD // 2, dtype=np.float64)
    theta = BASE ** (-2.0 * j / HD)
    ang = np.arange(S, dtype=np.float64)[:, None] * theta[None, :]
    cos = np.tile(np.cos(ang).T, (NH, 1)) * scale
    sin = np.tile(np.sin(ang).T, (NH, 1)) * scale
    return cos.astype(BF), sin.astype(BF)


def host_prep(consulta, chave, valor, Wq, bq, Wk, bk, Wv, bv, Wo, bo, S, DX, DO):
    """Build the 8 per-core input maps + metadata for unsharding."""
    perm = _perm_reim()
    cosq, sinq = _tables(S, 1.0 / np.sqrt(HD))
    cosk, sink = _tables(S, 1.0)
    qk_bias = bool(np.any(bq) or np.any(bk))

    xT = {}
    for b in range(consulta.shape[0]):
        xT[("xq", b)] = np.ascontiguousarray(consulta[b].T).astype(BF)
        xT[("xk", b)] = np.ascontiguousarray(chave[b].T).astype(BF)
        xT[("xv", b)] = np.ascontiguousarray(valor[b].T).astype(BF)

    DL = NH * HD
    in_maps = []
    n_cores = 8
    groups = 4  # head groups per batch
    for c in range(n_cores):
        b, hg = c // groups, c % groups
        rows = slice(DL * hg, DL * hg + DL)
        m = {
            "xq": xT[("xq", b)], "xk": xT[("xk", b)], "xv": xT[("xv", b)],
            "wq": np.ascontiguousarray(Wq[rows][perm].T).astype(BF),
            "wk": np.ascontiguousarray(Wk[rows][perm].T).astype(BF),
            "wv": np.ascontiguousarray(Wv[rows].T).astype(BF),
            "wo": np.ascontiguousarray(Wo[:, rows].T).astype(BF),
            "cosq": cosq, "sinq": sinq, "cosk": cosk, "sink": sink,
        }
        if qk_bias:
            m["bq"] = bq[rows][perm].astype(np.float32).reshape(DL, 1)
            m["bk"] = bk[rows][perm].astype(np.float32).reshape(DL, 1)
        in_maps.append(m)
    return in_maps, qk_bias


def assemble_output(results, bv, bo, Wo, S, DO):
    out = np.zeros((B, S, DO), dtype=np.float32)
    corr = (bv.astype(np.float32) @ Wo.astype(np.float32).T) + bo.astype(np.float32)
    for c in range(8):
        out[c // 4] += results[c]["out0"].astype(np.float32)
    out += corr[None, None, :]
    return out


_CACHE = {}


def kernel(consulta, chave, valor, Wq, bq, Wk, bk, Wv, bv, Wo, bo):
    import os

    args = [np.asarray(a, dtype=np.float32) for a in
            (consulta, chave, valor, Wq, bq, Wk, bk, Wv, bv, Wo, bo)]
    consulta, chave, valor, Wq, bq, Wk, bk, Wv, bv, Wo, bo = args
    S, DX, DO = consulta.shape[1], consulta.shape[2], Wo.shape[0]

    in_maps, qk_bias = host_prep(
        consulta, chave, valor, Wq, bq, Wk, bk, Wv, bv, Wo, bo, S, DX, DO
    )
    key = (S, DX, DO, qk_bias)
    if key not in _CACHE:
        _CACHE[key] = build_graph(S, DX, DO, qk_bias=qk_bias)
    nc = _CACHE[key]

    trace = bool(int(os.environ.get("KERNEL_TRACE", "0")))
    res = run_bass_kernel_spmd(nc, in_maps, core_ids=list(range(8)), trace=trace)
    if trace:
        kernel.last_exec_time_ns = res.exec_time_ns
        kernel.last_results = res
    return assemble_output(res.results, bv, bo, Wo, S, DO)



# revision 30
# speedup vs baseline: 1.0606x; 1.0606x over previous
"""Distributed multi-head attention (RoPE) kernel for 8 TRN2 NeuronCores.

Sharding: batch x head-group. Core c handles batch c//4 and heads
4*(c%4) .. 4*(c%4)+3 (4 of 16 heads). Each core computes its heads'
QKV projections (column-sharded), RoPE, attention, and a row-sharded
partial output projection; the host sums the 4 partials per batch.

Self-contained: only needs numpy/ml_dtypes + the concourse stack at
/opt/trn_rl_repo (the runtime environment of this container).
"""

import contextlib
import sys

for _p in ("/opt/trn_rl_repo",):
    if _p not in sys.path:
        sys.path.insert(0, _p)

import numpy as np
import ml_dtypes

import concourse.bass as bass
import concourse.mybir as mybir
import concourse.tile as tile
from concourse.bass_utils import run_bass_kernel_spmd

F32 = mybir.dt.float32
BF16 = mybir.dt.bfloat16
BF = ml_dtypes.bfloat16

B, S_FULL, D_FULL, H_FULL, HD = 2, 4096, 1024, 16, 64
NH = 4  # heads per core
BASE = 10000.0


def _patch_tile_drain():
    """The walrus build in this container rejects >1 sem wait on one CTRL
    instruction ("Too many sync wait commands"). Split the Tile exit
    drain's waits across multiple drain instructions."""
    from concourse.tile import ScopedClock, TileContext

    if getattr(TileContext, "_drain_patched", False):
        return
    MAXW = 1

    def _drain_and_barrier(self, tick_clock, wait_clock):
        nc = self.nc
        drain_inst = nc.sync.drain()
        wait_clock.add_sem_waits(
            drain_inst.ins, ScopedClock({None: tick_clock.global_clock})
        )
        si = drain_inst.ins.sync_info
        waits = list(si.on_wait)
        if len(waits) > MAXW:
            drain_inst.ins.sync_info = mybir.SyncInfo(
                on_wait=waits[:MAXW], on_update=list(si.on_update)
            )
            for i in range(MAXW, len(waits), MAXW):
                extra = nc.sync.drain()
                extra.ins.sync_info = mybir.SyncInfo(
                    on_wait=waits[i : i + MAXW], on_update=[]
                )
        nc.all_engine_barrier()
        assert self.sems is not None
        popped = nc._tile_sem_poison_stack.pop()
        assert popped is self._sem_poison
        nc.clear_and_free_semaphores(list(self.sems.allocated().values()))
        nc.all_engine_barrier()

    TileContext._drain_and_barrier = _drain_and_barrier

    # The same limit applies to every instruction type: split surplus sem
    # waits onto same-engine NoOps emitted just before the instruction.
    _orig_commit = TileContext._commit_and_lower
    _ctr = [0]

    def _commit_and_lower(self, inst, bb, old_bb_map, bb_to_exit_bb):
        si = getattr(inst, "sync_info", None)
        eng = getattr(inst, "engine", None)
        mx = (
            2
            if type(inst).__name__ in getattr(tile, "_WS_MAXW2_TYPES", ())
            else MAXW
        )
        if (
            si is not None
            and len(si.on_wait) > mx
            and eng is not None
            and eng != mybir.EngineType.Unassigned
        ):
            waits = list(si.on_wait)
            for w in waits[:-mx]:
                _ctr[0] += 1
                nop = mybir.InstNoOp(name=f"WS-{_ctr[0]}", ins=[], outs=[])
                nop.engine = eng
                nop.sync_info = mybir.SyncInfo(on_wait=[w], on_update=[])
                self.nc.register_instruction(nop, overwrite=True)
                bb.add_instruction(nop)
            inst.sync_info = mybir.SyncInfo(
                on_wait=waits[-mx:], on_update=list(si.on_update)
            )
        return _orig_commit(self, inst, bb, old_bb_map, bb_to_exit_bb)

    TileContext._commit_and_lower = _commit_and_lower
    TileContext._drain_patched = True


_patch_tile_drain()


WS_MAXW2_TYPES = ()


def build_graph(S, DX, DO, qk_bias=False):
    """One core's graph. Layouts (T = [dims, seq]):
      inputs xq/xk/xv: X.T [DX, S]; weights wq/wk: permuted-rows W.T [DX, 256]
      (columns ordered re(128)|im(128) across the 4 local heads);
      wv: W.T [DX, 256] natural head order; wo: Wo[:, cols].T [256, DO].
      cos/sin tables [128, S] (row 32*hl+j = freq j of local head hl);
      Q tables pre-scaled by 1/8 (the 1/sqrt(HD) softmax scale).
    Output "out": partial x @ Wo_c.T, [S, DO] bf16.
    """
    DL = NH * HD  # 256
    KT = DX // 128
    SGW = min(1024, S)
    QC = 512
    OC = min(512, DO)
    NSG = S // SGW
    NQB = S // QC
    NKT = S // 128
    NST = S // 128
    NI = NH * NQB
    MUL, ADD, SUB = mybir.AluOpType.mult, mybir.AluOpType.add, mybir.AluOpType.subtract

    tile._WS_MAXW2_TYPES = WS_MAXW2_TYPES
    nc = bass.Bass(target_bir_lowering=False)
    ext = {}
    for nm, shp, dt in [
        ("xq", [DX, S], BF16), ("xk", [DX, S], BF16), ("xv", [DX, S], BF16),
        ("wq", [DX, DL], BF16), ("wk", [DX, DL], BF16), ("wv", [DX, DL], BF16),
        ("wo", [DL, DO], BF16),
        ("cosq", [128, S], BF16), ("sinq", [128, S], BF16),
        ("cosk", [128, S], BF16), ("sink", [128, S], BF16),
    ]:
        ext[nm] = nc.declare_dram_parameter(nm, shp, dt, isOutput=False)
    if qk_bias:
        ext["bq"] = nc.declare_dram_parameter("bq", [DL, 1], F32, isOutput=False)
        ext["bk"] = nc.declare_dram_parameter("bk", [DL, 1], F32, isOutput=False)
    out_ext = nc.declare_dram_parameter("out0", [S, DO], BF16, isOutput=True)
    scratch = nc.dram_tensor("scratch_recip", [NI, QC], F32)
    scratch2 = nc.dram_tensor("scratch_sums", [NI, QC], F32)

    with tile.TileContext(nc) as tc, contextlib.ExitStack() as top:
        pers = top.enter_context(tc.tile_pool(name="pers", bufs=1))
        QTh = [pers.tile([128, S], BF16, tag=f"qt{i}", name=f"qt{i}") for i in range(2)]
        KTh = [pers.tile([128, S], BF16, tag=f"kt{i}", name=f"kt{i}") for i in range(2)]
        CXh = [pers.tile([128, S], BF16, tag=f"cx{i}", name=f"cx{i}") for i in range(2)]
        VP = [pers.tile([128, NH * 65], BF16, tag=f"vp{i}", name=f"vp{i}") for i in range(NST)]
        NTS = (NI + 3) // 4
        sum_t = [pers.tile([97, QC], F32, tag=f"sums{i}", name=f"sums{i}") for i in range(NTS)]
        for i in range(NTS):
            nc.vector.memset(sum_t[i][:, :], 1.0)
        wq_t = [pers.tile([128, DL], BF16, tag=f"wq{k}", name=f"wq{k}") for k in range(KT)]
        wk_t = [pers.tile([128, DL], BF16, tag=f"wk{k}", name=f"wk{k}") for k in range(KT)]
        wv_t = [pers.tile([128, DL], BF16, tag=f"wv{k}", name=f"wv{k}") for k in range(KT)]
        wo_t = [pers.tile([128, DO], BF16, tag=f"wo{k}", name=f"wo{k}") for k in range(DL // 128)]
        for k in range(KT):
            for w_t, nm in ((wv_t, "wv"), (wk_t, "wk"), (wq_t, "wq")):
                eng = nc.gpsimd if nm != "wv" else nc.sync
                eng.dma_start(
                    out=w_t[k][:, :], in_=ext[nm][k * 128 : (k + 1) * 128, :]
                )
        for k in range(DL // 128):
            nc.gpsimd.dma_start(
                out=wo_t[k][:, :], in_=ext["wo"][k * 128 : (k + 1) * 128, :]
            )
        bias_t = {}
        if qk_bias:
            for nm in ("bq", "bk"):
                bias_t[nm] = pers.tile([DL, 1], F32, tag=nm, name=nm+"_t")
                # [DL,1] spans 2 partition tiles; DMA per 128-partition block
                for m in range(DL // 128):
                    nc.sync.dma_start(
                        out=bias_t[nm][m * 128 : (m + 1) * 128, :],
                        in_=ext[nm][m * 128 : (m + 1) * 128, :],
                    )

        # ---- projections (V, then K, then Q) in one pool scope so Tile
        # can overlap across the phase boundaries; all projection psums share
        # one 4-slot ring of [128, SGW] tiles (8 banks). ----
        with tc.tile_pool(name="qkps", bufs=4, space="PSUM") as psqk, \
             tc.tile_pool(name="qksb", bufs=KT + 12) as sbs, \
             tc.tile_pool(name="tabs", bufs=1) as tabs, \
             tc.tile_pool(name="rope", bufs=2) as rp:
            psvp, vsb = psqk, sbs
            # PE warmup: dummy matmuls with no DMA deps run during the initial
            # input DMA wait, flipping HAM to K=8/8 before real matmuls start.
            wsb = tabs.tile([128, 512], BF16, tag="warm")
            nc.vector.memset(wsb[:, :], 0.0)
            wps = psqk.tile([128, 512], F32, tag="qkps", name="warm_ps",
                            padded_shape=[128, SGW])
            for _w in range(40):
                nc.tensor.matmul(
                    wps[:, :], wsb[:, 0:128], wsb[:, :],
                    start=True, stop=True,
                )
            # cos/sin tables: issue early so their 4 MB of DMA spreads across
            # the V-projection phase instead of colliding with Q/K x loads.
            tabt = {}
            for nm in ("cosq", "sinq", "cosk", "sink"):
                tabt[nm] = tabs.tile([128, S], BF16, tag=nm, name=nm+"_t")
                nc.gpsimd.dma_start(out=tabt[nm][:, :], in_=ext[nm][:, :])
            for stg in range(NSG):
                xs = []
                for k in range(KT):
                    t = vsb.tile([128, SGW], BF16, tag="xs")
                    eng = nc.sync if k % 2 == 0 else nc.scalar
                    eng.dma_start(
                        out=t[:, :],
                        in_=ext["xv"][k * 128 : (k + 1) * 128, stg * SGW : (stg + 1) * SGW],
                    )
                    xs.append(t)
                for st8 in range(SGW // 128):
                    st = stg * (SGW // 128) + st8
                    psv = psvp.tile([128, DL], F32, tag="qkps", name="psv", padded_shape=[128, SGW])
                    for k in range(KT):
                        nc.tensor.matmul(
                            psv[:, :],
                            xs[k][:, st8 * 128 : (st8 + 1) * 128],
                            wv_t[k][:, :],
                            start=(k == 0),
                            stop=(k == KT - 1),
                        )
                    vp3 = VP[st][:, :].rearrange("p (h c) -> p h c", c=65)
                    nc.vector.tensor_copy(
                        vp3[:, :, 0:64],
                        psv[:, :].rearrange("p (h c) -> p h c", c=64),
                    )
                    nc.vector.memset(vp3[:, :, 64:65], 1.0)

            # ---- Q/K projections + RoPE + head-major rearrange ----
            for xnm, w_t, dst, cnm, snm, bnm in (
                ("xq", wq_t, QTh, "cosq", "sinq", "bq"),
                ("xk", wk_t, KTh, "cosk", "sink", "bk"),
            ):
                cosT, sinT = tabt[cnm], tabt[snm]
                for sg in range(NSG):
                    ssl = slice(sg * SGW, (sg + 1) * SGW)
                    xs = []
                    for k in range(KT):
                        t = sbs.tile([128, SGW], BF16, tag="xs")
                        eng = nc.sync if k % 2 == 0 else nc.scalar
                        eng.dma_start(
                            out=t[:, :], in_=ext[xnm][k * 128 : (k + 1) * 128, ssl]
                        )
                        xs.append(t)
                    ps = [psqk.tile([128, SGW], F32, tag="qkps", name=f"qkps{_i}") for _i in range(2)]
                    for m in range(2):
                        for k in range(KT):
                            for qc in range(SGW // 512):
                                qsl = slice(qc * 512, (qc + 1) * 512)
                                nc.tensor.matmul(
                                    ps[m][:, qsl],
                                    w_t[k][:, m * 128 : (m + 1) * 128],
                                    xs[k][:, qsl],
                                    start=(k == 0),
                                    stop=(k == KT - 1),
                                )
                        if qk_bias:
                            nc.vector.tensor_scalar_add(
                                ps[m][:, :], ps[m][:, :],
                                bias_t[bnm][m * 128 : (m + 1) * 128, :],
                            )
                    a = rp.tile([128, SGW], F32, tag="ra")
                    b2 = rp.tile([128, SGW], F32, tag="rb")
                    ren = rp.tile([128, SGW], BF16, tag="ren")
                    imn = rp.tile([128, SGW], BF16, tag="imn")
                    nc.vector.tensor_tensor(out=a[:, :], in0=ps[0][:, :], in1=cosT[:, ssl], op=MUL)
                    nc.vector.tensor_tensor(out=b2[:, :], in0=ps[1][:, :], in1=sinT[:, ssl], op=MUL)
                    nc.vector.tensor_tensor(out=ren[:, :], in0=a[:, :], in1=b2[:, :], op=SUB)
                    nc.vector.tensor_tensor(out=a[:, :], in0=ps[0][:, :], in1=sinT[:, ssl], op=MUL)
                    nc.vector.tensor_tensor(out=b2[:, :], in0=ps[1][:, :], in1=cosT[:, ssl], op=MUL)
                    nc.vector.tensor_tensor(out=imn[:, :], in0=a[:, :], in1=b2[:, :], op=ADD)
                    # head-major rearrange via SBUF->SBUF DMA: pure partition
                    # moves, so they run on the DMA fabric instead of eating
                    # ~2.6us of DVE per (input, stage) in partition-sliced
                    # copies (a [32, S] copy costs as much as [128, S] on DVE).
                    for hl in range(NH):
                        pt, off = hl // 2, 64 * (hl % 2)
                        hsl = slice(32 * hl, 32 * hl + 32)
                        nc.gpsimd.dma_start(
                            out=dst[pt][off : off + 32, ssl], in_=ren[hsl, :]
                        )
                        nc.gpsimd.dma_start(
                            out=dst[pt][off + 32 : off + 64, ssl], in_=imn[hsl, :]
                        )

        # ---- attention + incremental normalization + per-hp O-projection.
        # scoresT = K_h @ Q_h.T per 128-k-tile (the two heads' matmuls run
        # concurrently via row-group tiling); exp on ACT for 20 of every 32
        # k-tiles and bf16-Schraudolph on DVE for the other 12 (the attention
        # phase is otherwise ACT-exp serialized); outT[h] accumulates
        # [V_h|1].T @ Pexp over k. After each (hp, qb): stage ctxT, reciprocal
        # the sums row, DMA round-trip broadcast it and normalize in place.
        # After both hp: the partial output projection ctx @ Wo_c -> out.
        # Schraudolph bf16: bits16 = round(x * 128/ln2 + (16256 - 7.25));
        # DVE f32->int16 convert is round-to-nearest (verified on HW), the
        # -7.25 centers the relative error at zero so softmax denominators
        # cancel most of it. RMS rel err ~1.8% on the DVE half of tiles.
        #
        # Software pipeline: whole-tile exp ([128, 2*QC], both heads in one
        # instruction) alternating between ACT and the DVE (bf16 Schraudolph,
        # 7 of every 16 slots) -- the ~185ns bank-free -> scores-refill ->
        # exp handoff is paid per exp INSTRUCTION, so whole-tile halves it
        # versus per-head exps. The scores pair for slot i+2 is issued
        # BEFORE slot i's AV matmuls: both unblock when exp(i) frees its
        # banks, and scores-first ordering lets exp(i+2) start ~650ns
        # earlier -- with plain ordering the chain
        # exp(i) -> AV(i) -> scores(i+2) -> exp(i+2) serializes each exp
        # engine behind Tensor work. Normalization for a finished qb is
        # deferred into the next qb and spread over 4 slots, with all its
        # DMAs on the (attention-idle) sync queue, so its round-trip waits
        # never head-of-line-block the strict-FIFO DVE queue.
        SCH_A16 = 184.6649652337873  # 2^7 / ln 2
        SCH_B16 = 16256.0 - 7.25
        DVE_I16 = (1, 3, 5, 7, 9, 11, 13)  # i%16 slots whose exp runs on DVE
        with tc.tile_pool(name="aps", bufs=2, space="PSUM") as aps, \
             tc.tile_pool(name="ops", bufs=4, space="PSUM") as ops, \
             tc.tile_pool(name="pex", bufs=6) as pex, \
             tc.tile_pool(name="nrm", bufs=4) as nrm, \
             tc.tile_pool(name="osb", bufs=4) as osb:
            flat = [
                (hp, qb, kt)
                for hp in range(2)
                for qb in range(NQB)
                for kt in range(NKT)
            ]
            pso_of = {}
            pss_q = {}

            def emit_scores(i):
                hp, qb, kt = flat[i]
                qsl = slice(qb * QC, (qb + 1) * QC)
                ksl = slice(kt * 128, (kt + 1) * 128)
                pss = aps.tile([128, 2 * QC], F32, tag="scoresT")
                for h in range(2):
                    hoff = 64 * h
                    nc.tensor.matmul(
                        pss[:, h * QC : (h + 1) * QC],
                        KTh[hp][hoff : hoff + 64, ksl],
                        QTh[hp][hoff : hoff + 64, qsl],
                        start=True,
                        stop=True,
                    )
                pss_q[i] = pss

            def norm_phase1(hp, qb, state):
                # stage the exp-sum rows out of PSUM and kick the DMA chain
                # (scratch2 write then r4 read are FIFO-ordered on gpsimd);
                # h0's copy on Scalar, h1's on Vector to balance the load
                pso = pso_of[(hp, qb)]
                r4s = []
                for h in range(2):
                    hl = 2 * hp + h
                    ins = hl * NQB + qb
                    r = 32 * (ins % 4)
                    srow = sum_t[ins // 4][r : r + 1, :]
                    if h == 0:
                        nc.scalar.copy(srow, pso[h][64:65, :])
                    else:
                        nc.vector.tensor_copy(srow, pso[h][64:65, :])
                    nc.sync.dma_start(out=scratch2[ins : ins + 1, :], in_=srow)
                    r4 = nrm.tile([128, QC // 128], F32, tag="r4")
                    nc.sync.dma_start(
                        out=r4[:, :],
                        in_=scratch2[ins : ins + 1, :].rearrange(
                            "1 (p f) -> p f", p=128
                        ),
                    )
                    r4s.append(r4)
                state["r4"] = r4s

            def norm_phase2(hp, qb, state):
                for h in range(2):
                    hl = 2 * hp + h
                    ins = hl * NQB + qb
                    rc4 = nrm.tile([128, QC // 128], F32, tag="rc4")
                    nc.vector.reciprocal(rc4[:, :], state["r4"][h][:, :])
                    nc.sync.dma_start(
                        out=scratch[ins : ins + 1, :].rearrange(
                            "1 (p f) -> p f", p=128
                        ),
                        in_=rc4[:, :],
                    )
                bc = nrm.tile([128, QC], F32, tag="bc")
                for h in range(2):
                    hl = 2 * hp + h
                    ins = hl * NQB + qb
                    nc.sync.dma_start(
                        out=bc[64 * h : 64 * h + 64, :],
                        in_=scratch[ins : ins + 1, :].broadcast_to([64, QC]),
                    )
                state["bc"] = bc

            def norm_phase3(hp, qb, state, h):
                qsl = slice(qb * QC, (qb + 1) * QC)
                pso = pso_of[(hp, qb)]
                bc = state["bc"]
                nc.vector.tensor_tensor(
                    out=CXh[hp][64 * h : 64 * h + 64, qsl],
                    in0=pso[h][0:64, :],
                    in1=bc[64 * h : 64 * h + 64, :],
                    op=MUL,
                )
                if h == 1:
                    pso_of.pop((hp, qb))

            emit_scores(0)
            emit_scores(1)
            prev_qb = None
            norm_state = {}
            for i, (hp, qb, kt) in enumerate(flat):
                if kt == 0:
                    pso_of[(hp, qb)] = [
                        ops.tile([65, QC], F32, tag="outT", name=f"outT{_i}")
                        for _i in range(2)
                    ]
                if i + 2 < len(flat):
                    emit_scores(i + 2)
                pss = pss_q.pop(i)
                pe = pex.tile([128, 2 * QC], BF16, tag="pexp")
                if i % 16 in DVE_I16:
                    nc.vector.tensor_scalar(
                        pe[:, :].bitcast(mybir.dt.int16), pss[:, :],
                        SCH_A16, SCH_B16,
                        op0=MUL, op1=ADD,
                    )
                else:
                    nc.scalar.activation(
                        pe[:, :], pss[:, :],
                        mybir.ActivationFunctionType.Exp,
                    )
                for h in range(2):
                    hl = 2 * hp + h
                    nc.tensor.matmul(
                        pso_of[(hp, qb)][h][:, :],
                        VP[kt][:, 65 * hl : 65 * hl + 65],
                        pe[:, h * QC : (h + 1) * QC],
                        start=(kt == 0),
                        stop=(kt == NKT - 1),
                    )
                # deferred normalization of the previous qb
                # norm phases land on ACT-exp slots (even kt) so their DVE ops
                # never share a slot with a DVE-exp
                if prev_qb is not None:
                    if kt == 2:
                        norm_phase1(*prev_qb, norm_state)
                    elif kt == 6:
                        norm_phase2(*prev_qb, norm_state)
                    elif kt == 8:
                        norm_phase3(*prev_qb, norm_state, 0)
                    elif kt == 10:
                        norm_phase3(*prev_qb, norm_state, 1)
                if kt == NKT - 1:
                    prev_qb = (hp, qb)
            norm_phase1(*prev_qb, norm_state)
            norm_phase2(*prev_qb, norm_state)
            norm_phase3(*prev_qb, norm_state, 0)
            norm_phase3(*prev_qb, norm_state, 1)
            # O-projection: out = ctx @ Wo_c.T, accumulating both head
            # pairs in PSUM -> single output. mt-outer so only the final
            # s-tiles wait on the last qb's normalization; both DO-halves of
            # an s-tile merge into one SBUF tile -> one out DMA per s-tile.
            # PSUM->SBUF staging alternates Vector/Scalar (the tail is
            # staging-bound, both engines are idle here); out DMA alternates
            # sync/gpsimd queues.
            for mt in range(NST):
                ob = osb.tile([128, DO], BF16, tag="ob")
                for n in range(DO // OC):
                    nsl = slice(n * OC, (n + 1) * OC)
                    po = ops.tile([128, OC], F32, tag="outT", name="po")
                    for k2 in range(2):
                        nc.tensor.matmul(
                            po[:, :],
                            CXh[k2][:, mt * 128 : (mt + 1) * 128],
                            wo_t[k2][:, nsl],
                            start=(k2 == 0),
                            stop=(k2 == 1),
                        )
                    if n % 2 == 0:
                        nc.vector.tensor_copy(ob[:, nsl], po[:, :])
                    else:
                        nc.scalar.copy(ob[:, nsl], po[:, :])
                eng = nc.sync if mt % 2 == 0 else nc.gpsimd
                eng.dma_start(
                    out=out_ext[mt * 128 : (mt + 1) * 128, :],
                    in_=ob[:, :],
                )
    return nc


def _perm_reim():
    """Permutation of a 256-row local head block: [re rows (128), im rows]."""
    re = [64 * hl + 2 * j for hl in range(NH) for j in range(HD // 2)]
    im = [64 * hl + 2 * j + 1 for hl in range(NH) for j in range(HD // 2)]
    return np.array(re + im)


def _tables(S, scale):
    j = np.arange(HD // 2, dtype=np.float64)
    theta = BASE ** (-2.0 * j / HD)
    ang = np.arange(S, dtype=np.float64)[:, None] * theta[None, :]
    cos = np.tile(np.cos(ang).T, (NH, 1)) * scale
    sin = np.tile(np.sin(ang).T, (NH, 1)) * scale
    return cos.astype(BF), sin.astype(BF)


def host_prep(consulta, chave, valor, Wq, bq, Wk, bk, Wv, bv, Wo, bo, S, DX, DO):
    """Build the 8 per-core input maps + metadata for unsharding."""
    perm = _perm_reim()
    cosq, sinq = _tables(S, 1.0 / np.sqrt(HD))
    cosk, sink = _tables(S, 1.0)
    qk_bias = bool(np.any(bq) or np.any(bk))

    xT = {}
    for b in range(consulta.shape[0]):
        xT[("xq", b)] = np.ascontiguousarray(consulta[b].T).astype(BF)
        xT[("xk", b)] = np.ascontiguousarray(chave[b].T).astype(BF)
        xT[("xv", b)] = np.ascontiguousarray(valor[b].T).astype(BF)

    DL = NH * HD
    in_maps = []
    n_cores = 8
    groups = 4  # head groups per batch
    for c in range(n_cores):
        b, hg = c // groups, c % groups
        rows = slice(DL * hg, DL * hg + DL)
        m = {
            "xq": xT[("xq", b)], "xk": xT[("xk", b)], "xv": xT[("xv", b)],
            "wq": np.ascontiguousarray(Wq[rows][perm].T).astype(BF),
            "wk": np.ascontiguousarray(Wk[rows][perm].T).astype(BF),
            "wv": np.ascontiguousarray(Wv[rows].T).astype(BF),
            "wo": np.ascontiguousarray(Wo[:, rows].T).astype(BF),
            "cosq": cosq, "sinq": sinq, "cosk": cosk, "sink": sink,
        }
        if qk_bias:
            m["bq"] = bq[rows][perm].astype(np.float32).reshape(DL, 1)
            m["bk"] = bk[rows][perm].astype(np.float32).reshape(DL, 1)
        in_maps.append(m)
    return in_maps, qk_bias


def assemble_output(results, bv, bo, Wo, S, DO):
    out = np.zeros((B, S, DO), dtype=np.float32)
    corr = (bv.astype(np.float32) @ Wo.astype(np.float32).T) + bo.astype(np.float32)
    for c in range(8):
        out[c // 4] += results[c]["out0"].astype(np.float32)
    out += corr[None, None, :]
    return out


_CACHE = {}


def kernel(consulta, chave, valor, Wq, bq, Wk, bk, Wv, bv, Wo, bo):
    import os

    args = [np.asarray(a, dtype=np.float32) for a in
            (consulta, chave, valor, Wq, bq, Wk, bk, Wv, bv, Wo, bo)]
    consulta, chave, valor, Wq, bq, Wk, bk, Wv, bv, Wo, bo = args
    S, DX, DO = consulta.shape[1], consulta.shape[2], Wo.shape[0]

    in_maps, qk_bias = host_prep(
        consulta, chave, valor, Wq, bq, Wk, bk, Wv, bv, Wo, bo, S, DX, DO
    )
    key = (S, DX, DO, qk_bias)
    if key not in _CACHE:
        _CACHE[key] = build_graph(S, DX, DO, qk_bias=qk_bias)
    nc = _CACHE[key]

    trace = bool(int(os.environ.get("KERNEL_TRACE", "0")))
    res = run_bass_kernel_spmd(nc, in_maps, core_ids=list(range(8)), trace=trace)
    if trace:
        kernel.last_exec_time_ns = res.exec_time_ns
        kernel.last_results = res
    return assemble_output(res.results, bv, bo, Wo, S, DO)

